# revision 20
# baseline (speedup 1.0000x reference)
"""Trainium2 Bass kernel for nn_AE_spikes (spiking autoencoder, 784-128-128-128-784).

Algorithm restructure (mathematically equivalent to the reference spiking net):
- Identity bin scaling (all 5 bin arrays equal) -> weights used as-is.
- Input layer digitize + integrate-and-fire has a closed form for the
  cumulative spike count: F_k = round(k*a - 35/64), a = max(floor(16 f), 1)/16.
- Each layer's matmul consumes the CUMULATIVE spike counts C of the previous
  layer, giving the cumulative drive D_k = W @ C_k directly.
- The integrate-and-fire recurrence is the max-chase C_k = max(C_{k-1},
  round(D_k + b - 1/2)), run as ONE DVE tensor_tensor_scan per batch lane with
  17-slot chains (16 steps + one dummy slot resetting the scan state).
- The output layer needs no scan: relu(round(max_k G_k)) via a max-reduce.

v7 performance structure (on top of v4; measured ~29.8us vs the 32.1us v4
baseline at the fast chip clock):
- ALL round-to-integer ops ride the f16 downcast-on-write (RNE; ULP=1 in
  [1024, 2048)) instead of dedicated DVE magic-number rounds, exploiting
  round(max(x)) == max(round(x)) (monotonicity):
  * F' = F + 1536 comes straight out of the 16 per-k multiplies (zone
    [1535.4, 1552] is entirely ULP=1) -- the 7 separate round ops are gone.
  * Hidden layers: ACT writes psum -> gc f16 with per-partition bias
    (+1024 offset and the 0.5 shift and the offset-rowsum correction all
    folded in); the f16 write rounds. Values below 1024 mis-round on the
    0.5 grid but can never win the scan max against state >= 1024.
  * The scan runs (op0=max, op1=min) with initial=1024; data1 is 65504
    everywhere and 1024 at each chain's dummy slot, which resets state to
    exactly 1024 (state >= 1024 inductively). Per hidden layer the chain
    is just matmul -> ACT -> scan (the v4 DVE round op is gone).
  * Layer 3: ACT copies psum -> H f16 (+bias, rounds), then a DVE
    max-reduce; relu and the -1024 land on the host (commute with max).
- The offset shifts ride the matmul: W @ (C + OFF) = W@C + OFF*rowsum(W);
  host subtracts OFF*rowsum(W_f16) from the bias columns.
- feat DMA halves issue on the sync + scalar queues in parallel (the ~2.3us
  DMA startup latency dominates the head; parallel issue lands both halves
  together ~0.8us earlier than serial issue).
- F multiplies split DVE(k=1..12, ~130ns effective back-to-back) /
  ACT(k=13..16, ~470ns each) so both engines finish together. GPSIMD is
  kept off the F path: its SBUF port contends with DVE and slows both.
- No PE warmup fillers: measured on this part, the HAM clock gate never
  releases (5.5us of continuous PE busy still left every matmul at
  1.2 GHz), so fillers only delayed real work.

Sharding: pure data-parallel over the batch (256 -> 32 images per core), all
weights replicated, no collectives. Host pre-transposes weights/features,
folds all bias/offset corrections, and applies relu(x-1024)/16 on the way out.
"""
import sys

if "/opt/trn_rl_repo" not in sys.path:
    sys.path.insert(0, "/opt/trn_rl_repo")

import numpy as np

IN, HID, NS, NB = 784, 128, 16, 32  # in-dim, hidden, steps, batch per core
PCH, NCH = 112, 7                   # pixel-partition chunking: 784 = 112 * 7
SLOT = NS + 1                       # 17-slot chains (dummy slot resets scan state)
HB = NB // 2                        # 16 images per lane
NCORES = 8
C35_64 = 0.546875                   # 35/64: exact floor shift for 1/16-grid values
F16OFF = 1536.0                     # f16 ULP=1 magnitude for the digitize trick
OFF_F = 1536.0                      # +offset carried by F' (input spike counts)
OFF_C = 1024.0                      # +offset carried by C' (hidden counts) and Mx
FMAX16 = 65504.0                    # f16 max: scan data1 "no-op" value
FEAT_SPLIT = 4                      # feat DMA split: chunks [0:4) then [4:7)
F_DVE = list(range(1, 13))          # F-multiply engine split (no GPSIMD: the
F_ACT = list(range(13, 17))         # Pool SBUF port contends with DVE)

_CACHE = {}


def _build():
    import concourse.bacc as bacc
    import concourse.mybir as mybir
    from concourse import tile

    f32, f16 = mybir.dt.float32, mybir.dt.float16
    A = mybir.AluOpType
    ACT_ID = mybir.ActivationFunctionType.Identity

    nc = bacc.Bacc("TRN2", target_bir_lowering=False, debug=False)

    feat_e = nc.dram_tensor("feat", [PCH, NCH, NB], f32, kind="ExternalInput").ap()
    w0_e = nc.dram_tensor("w0T", [PCH, NCH, HID], f16, kind="ExternalInput").ap()
    w1_e = nc.dram_tensor("w1T", [HID, HID], f16, kind="ExternalInput").ap()
    w2_e = nc.dram_tensor("w2T", [HID, HID], f16, kind="ExternalInput").ap()
    w3_e = nc.dram_tensor("w3T", [HID, NCH, PCH], f16, kind="ExternalInput").ap()
    # biases+corrections, one f32 tensor: col 0..2 = layers 0..2 [128];
    # cols 3..9 = layer-3 chunks [112, 7] (rows 112+ pad)
    bt_e = nc.dram_tensor("bt", [HID, 3 + NCH], f32, kind="ExternalInput").ap()
    out_e = nc.dram_tensor("out", [PCH, NCH, NB], f16, kind="ExternalOutput").ap()

    with tile.TileContext(nc) as tc:
        with (
            tc.tile_pool(name="sbuf", bufs=1) as sb,
            tc.tile_pool(name="psumh", bufs=1, space="PSUM") as psh,
            tc.tile_pool(name="psum3", bufs=6, space="PSUM") as ps3,
        ):
            # ---- loads (host pre-transposed; all partition-contiguous) ----
            feat = sb.tile([PCH, NCH, NB], f32, tag="feat")
            nc.sync.dma_start(feat[:, 0:FEAT_SPLIT, :], feat_e[:, 0:FEAT_SPLIT, :])
            # feat second half rides the scalar queue FIRST (parallel DMA
            # startup with featA); then bt + w0 (short) so the ACT engine is
            # free for its F-multiply share; the rest on sync behind featA.
            nc.scalar.dma_start(
                feat[:, FEAT_SPLIT:NCH, :], feat_e[:, FEAT_SPLIT:NCH, :]
            )
            bt = sb.tile([HID, 3 + NCH], f32, tag="bt")
            nc.scalar.dma_start(bt[:], bt_e[:])
            w0s = sb.tile([PCH, NCH, HID], f16, tag="w0")
            nc.scalar.dma_start(w0s[:], w0_e[:])
            w1s = sb.tile([HID, HID], f16, tag="w1")
            nc.sync.dma_start(w1s[:], w1_e[:])
            w2s = sb.tile([HID, HID], f16, tag="w2")
            nc.sync.dma_start(w2s[:], w2_e[:])
            w3s = sb.tile([HID, NCH, PCH], f16, tag="w3")
            nc.sync.dma_start(w3s[:], w3_e[:])

            b0s = bt[:, 0:1]
            b1s = bt[:, 1:2]
            b2s = bt[:, 2:3]

            # ---- constants (Pool memsets; all early, off critical path) ----
            # scan data1: 65504 pass-through, 1024 at each chain's dummy slot
            maskR = sb.tile([HID, HB, SLOT], f16, tag="maskR")
            nc.gpsimd.memset(maskR[:], FMAX16)
            nc.gpsimd.memset(maskR[:, :, NS:SLOT], OFF_C)
            # gc tiles: dummy slot must hold a harmless finite value (0)
            gc_tiles = {}
            for lname in ("00", "01", "10", "11", "20", "21"):
                g = sb.tile([HID, HB, SLOT], f16, tag=f"gc{lname}")
                nc.gpsimd.memset(g[:, :, NS:SLOT], 0.0)
                gc_tiles[lname] = g
            # per-k F-multiply constants for the ACT ops (activation bias must
            # be an AP; the const-AP pool only carries 0.0/1.0)
            fbias = sb.tile([PCH, len(F_ACT)], f32, tag="fbias")
            for i, k in enumerate(F_ACT):
                nc.gpsimd.memset(fbias[:, i : i + 1], OFF_F - 96.0 * k - C35_64)

            # ---- digitize: 2 DVE ops via f16-at-1536 ----
            # t = f16(16 f - 0.5 + 2^-17 + 1536) == m + 1536 (integer, exact)
            # a' = max(t, 1537)/16 = a + 96 (exact on the 1/16 grid in f16)
            t1 = sb.tile([PCH, NCH, NB], f16, tag="dig1")
            a16 = sb.tile([PCH, NCH, NB], f16, tag="a16")
            for lo, hi in ((0, FEAT_SPLIT), (FEAT_SPLIT, NCH)):
                nc.vector.tensor_scalar(
                    t1[:, lo:hi, :], feat[:, lo:hi, :],
                    16.0, -0.5 + 2.0 ** -17 + F16OFF, A.mult, A.add,
                )
            nc.vector.tensor_scalar(
                a16[:], t1[:], F16OFF + 1.0, 1.0 / 16.0, A.max, A.mult,
            )

            # ---- F' multiplies: F'_k = f16(k*a' + (OFF_F - 96k - 35/64))
            # = round(k*a - 35/64) + OFF_F exactly (zone [1535.4, 1552] has
            # f16 ULP=1; fp32 pre-write value exact on the 1/64 grid).
            # Split across DVE / ACT / GPSIMD.
            F = sb.tile([PCH, NS, NCH, NB], f16, tag="F")

            def f_const(k):
                return OFF_F - 96.0 * k - C35_64

            for k in F_DVE:
                nc.vector.tensor_scalar(
                    F[:, k - 1], a16[:], float(k), f_const(k), A.mult, A.add
                )
            for i, k in enumerate(F_ACT):
                nc.scalar.activation(
                    F[:, k - 1], a16[:], ACT_ID,
                    bias=fbias[:, i : i + 1], scale=float(k),
                )

            # ---- layer 0: D0' = W0 @ F' per lane, psum (k, j); chunk-major
            # accumulation; one group per lane so lane 0 closes early ----
            D0L = []
            for ln in range(2):
                D = psh.tile([HID, HB * NS], f32, tag=f"dh{ln}")
                for c in range(NCH):
                    nc.tensor.matmul(
                        D[:],
                        w0s[:, c, :],
                        F[:, :, c, ln * HB : (ln + 1) * HB],
                        start=(c == 0),
                        stop=(c == NCH - 1),
                    )
                D0L.append(D)

            def act_scan(din, bias, lname):
                # ACT: psum -> gc f16 in chain order; the f16 write rounds
                # (bias carries +1024 and all corrections). Then the scan:
                # state = min(max(gc, state), data1); reset to 1024 at dummies.
                gc = gc_tiles[lname]
                nc.scalar.activation(gc[:, :, 0:NS], din, ACT_ID, bias=bias, scale=1.0)
                C = sb.tile([HID, HB, SLOT], f16, tag=f"C{lname}")
                nc.vector.tensor_tensor_scan(
                    C[:].rearrange("p j s -> p (j s)"),
                    gc[:].rearrange("p j s -> p (j s)"),
                    maskR[:].rearrange("p j s -> p (j s)"),
                    OFF_C,
                    A.max,
                    A.min,
                )
                return C

            C0 = [None, None]
            for ln in range(2):
                # D0 psum is (k, j): ACT reads it permuted to (j, k)
                C0[ln] = act_scan(
                    D0L[ln][:].rearrange("p (k j) -> p j k", j=HB), b0s, f"0{ln}"
                )

            def hidden_lane(Cin, w, bias, lname, tag):
                D = psh.tile([HID, HB * NS], f32, tag=tag)
                nc.tensor.matmul(D[:], w[:], Cin[:, :, 0:NS], start=True, stop=True)
                # psum is already (j, k) = chain order
                return act_scan(
                    D[:].rearrange("p (j k) -> p j k", k=NS), bias, lname
                )

            C1 = [None, None]
            C2 = [None, None]
            for ln in range(2):
                C1[ln] = hidden_lane(C0[ln], w1s, b1s, f"1{ln}", f"dh{ln}")
            for ln in range(2):
                C2[ln] = hidden_lane(C1[ln], w2s, b2s, f"2{ln}", f"dh{ln}")

            # ---- layer 3 (output): per chunk, two lane matmuls -> ACT copies
            # psum -> H f16 (+bias, the f16 write rounds) -> DVE max-reduce.
            # round/relu/-1024 all commute with the max; relu on host. ----
            H = sb.tile([PCH, NCH, NB, NS], f16, tag="H")
            Mx = sb.tile([PCH, NCH, NB], f16, tag="mx")
            D3L = []
            for c in range(NCH):
                D3 = ps3.tile([PCH, NB * NS], f32, tag="d3")
                nc.tensor.matmul(
                    D3[:, 0 : HB * NS], w3s[:, c, :], C2[0][:, :, 0:NS],
                    start=True, stop=True,
                )
                D3L.append(D3)
            for c in range(NCH):
                D3 = D3L[c]
                nc.tensor.matmul(
                    D3[:, HB * NS : NB * NS], w3s[:, c, :], C2[1][:, :, 0:NS],
                    start=True, stop=True,
                )
                nc.scalar.activation(
                    H[:, c],
                    D3[:].rearrange("p (j k) -> p j k", k=NS),
                    ACT_ID,
                    bias=bt[0:PCH, 3 + c : 4 + c],
                    scale=1.0,
                )
                nc.vector.tensor_reduce(
                    Mx[:, c, :], H[:, c], mybir.AxisListType.X, A.max
                )

            nc.sync.dma_start(out_e[:, 0:4, :], Mx[:, 0:4, :])
            nc.sync.dma_start(out_e[:, 4:NCH, :], Mx[:, 4:NCH, :])

    nc.compile()
    return nc


def _get_nc():
    if "nc" not in _CACHE:
        _CACHE["nc"] = _build()
    return _CACHE["nc"]


def _prep_in_maps(features, W0, b0, W1, b1, W2, b2, W3, b3):
    f32, f16 = np.float32, np.float16
    w0h = W0.astype(f16)
    w1h = W1.astype(f16)
    w2h = W2.astype(f16)
    w3h = W3.astype(f16)
    w0T = np.ascontiguousarray(
        w0h.T.reshape(NCH, PCH, HID).transpose(1, 0, 2)
    )  # [112, 7, 128]; w0T[p,c,m] = W0[m, c*112+p]
    w1T = np.ascontiguousarray(w1h.T)
    w2T = np.ascontiguousarray(w2h.T)
    w3T = np.ascontiguousarray(w3h.T.reshape(HID, NCH, PCH))

    # bias columns: b - 1/2 + OFF_C - OFF_rhs * rowsum(W_f16), in f64 -> f32.
    # (the 2^-18 ceil-strictness delta of v4 is below the f32 ULP at 1024 and
    # is dropped; threshold-exact cases are measure-zero in fp32.)
    def bias_col(b, wh, off_rhs):
        rs = wh.astype(np.float64).sum(axis=1)
        return b.astype(np.float64) - 0.5 + OFF_C - off_rhs * rs

    bt = np.zeros((HID, 3 + NCH), dtype=f32)
    bt[:, 0] = bias_col(b0, w0h, OFF_F).astype(f32)
    bt[:, 1] = bias_col(b1, w1h, OFF_C).astype(f32)
    bt[:, 2] = bias_col(b2, w2h, OFF_C).astype(f32)
    bt[0:PCH, 3:] = bias_col(b3, w3h, OFF_C).astype(f32).reshape(NCH, PCH).T

    in_maps = []
    for i in range(NCORES):
        shard = features[i * NB : (i + 1) * NB].astype(f32)  # [32, 784]
        feat = np.ascontiguousarray(
            shard.reshape(NB, NCH, PCH).transpose(2, 1, 0)
        )  # [112, 7, 32]
        in_maps.append(
            {
                "feat": feat,
                "w0T": w0T,
                "w1T": w1T,
                "w2T": w2T,
                "w3T": w3T,
                "bt": bt,
            }
        )
    return in_maps


def _assemble(results):
    outs = []
    for i in range(NCORES):
        o = results[i]["out"].astype(np.float32)
        # host: relu(x - 1024)/16 (commutes with the on-device max-reduce)
        o = np.maximum(o - np.float32(OFF_C), 0.0) * np.float32(1.0 / 16.0)
        outs.append(o.transpose(2, 1, 0).reshape(NB, IN))  # [32, 784]
    return np.concatenate(outs, axis=0)


def kernel(features, W0, b0, W1, b1, W2, b2, W3, b3, _trace=False):
    import time
    from concourse.bass_utils import run_bass_kernel_spmd

    nc = _get_nc()
    in_maps = _prep_in_maps(features, W0, b0, W1, b1, W2, b2, W3, b3)
    # The axon-tunneled device occasionally reports a transient
    # NRT_EXEC_UNIT_UNRECOVERABLE on the first attempt after a prior process
    # exited; it recovers on retry.
    last_exc = None
    for attempt in range(3):
        try:
            res = run_bass_kernel_spmd(nc, in_maps, list(range(NCORES)), trace=_trace)
            break
        except Exception as e:  # noqa: BLE001
            last_exc = e
            time.sleep(10 * (attempt + 1))
    else:
        raise last_exc
    out = _assemble(res.results)
    if _trace:
        _CACHE["last_result"] = res
    return out


# revision 21
# speedup vs baseline: 1.0063x; 1.0063x over previous
"""Trainium2 Bass kernel for nn_AE_spikes (spiking autoencoder, 784-128-128-128-784).

Algorithm restructure (mathematically equivalent to the reference spiking net):
- Identity bin scaling (all 5 bin arrays equal) -> weights used as-is.
- Input layer digitize + integrate-and-fire has a closed form for the
  cumulative spike count: F_k = round(k*a - 35/64), a = max(floor(16 f), 1)/16.
- Each layer's matmul consumes the CUMULATIVE spike counts C of the previous
  layer, giving the cumulative drive D_k = W @ C_k directly.
- The integrate-and-fire recurrence is the max-chase C_k = max(C_{k-1},
  round(D_k + b - 1/2)), run as ONE DVE tensor_tensor_scan per batch lane with
  17-slot chains (16 steps + one dummy slot resetting the scan state).
- The output layer needs no scan: relu(round(max_k G_k)) via a max-reduce.

v7 performance structure (on top of v4; measured ~29.8us vs the 32.1us v4
baseline at the fast chip clock):
- ALL round-to-integer ops ride the f16 downcast-on-write (RNE; ULP=1 in
  [1024, 2048)) instead of dedicated DVE magic-number rounds, exploiting
  round(max(x)) == max(round(x)) (monotonicity):
  * F' = F + 1536 comes straight out of the 16 per-k multiplies (zone
    [1535.4, 1552] is entirely ULP=1) -- the 7 separate round ops are gone.
  * Hidden layers: ACT writes psum -> gc f16 with per-partition bias
    (+1024 offset and the 0.5 shift and the offset-rowsum correction all
    folded in); the f16 write rounds. Values below 1024 mis-round on the
    0.5 grid but can never win the scan max against state >= 1024.
  * The scan runs (op0=max, op1=min) with initial=1024; data1 is 65504
    everywhere and 1024 at each chain's dummy slot, which resets state to
    exactly 1024 (state >= 1024 inductively). Per hidden layer the chain
    is just matmul -> ACT -> scan (the v4 DVE round op is gone).
  * Layer 3: ACT copies psum -> H f16 (+bias, rounds), then a DVE
    max-reduce; relu and the -1024 land on the host (commute with max).
- The offset shifts ride the matmul: W @ (C + OFF) = W@C + OFF*rowsum(W);
  host subtracts OFF*rowsum(W_f16) from the bias columns.
- feat DMA halves issue on the sync + scalar queues in parallel (the ~2.3us
  DMA startup latency dominates the head; parallel issue lands both halves
  together ~0.8us earlier than serial issue).
- F multiplies split DVE(k=1..12, ~130ns effective back-to-back) /
  ACT(k=13..16, ~470ns each) so both engines finish together. GPSIMD is
  kept off the F path: its SBUF port contends with DVE and slows both.
- No PE warmup fillers: measured on this part, the HAM clock gate never
  releases (5.5us of continuous PE busy still left every matmul at
  1.2 GHz), so fillers only delayed real work.

Sharding: pure data-parallel over the batch (256 -> 32 images per core), all
weights replicated, no collectives. Host pre-transposes weights/features,
folds all bias/offset corrections, and applies relu(x-1024)/16 on the way out.
"""
import sys

if "/opt/trn_rl_repo" not in sys.path:
    sys.path.insert(0, "/opt/trn_rl_repo")

import numpy as np

IN, HID, NS, NB = 784, 128, 16, 32  # in-dim, hidden, steps, batch per core
PCH, NCH = 112, 7                   # pixel-partition chunking: 784 = 112 * 7
SLOT = NS + 1                       # 17-slot chains (dummy slot resets scan state)
HB = NB // 2                        # 16 images per lane
NCORES = 8
C35_64 = 0.546875                   # 35/64: exact floor shift for 1/16-grid values
F16OFF = 1536.0                     # f16 ULP=1 magnitude for the digitize trick
OFF_F = 1536.0                      # +offset carried by F' (input spike counts)
OFF_C = 1024.0                      # +offset carried by C' (hidden counts) and Mx
FMAX16 = 65504.0                    # f16 max: scan data1 "no-op" value
FEAT_SPLIT = 4                      # feat DMA split: chunks [0:4) then [4:7)
F_DVE = list(range(1, 15))          # F-multiply engine split (no GPSIMD: the
F_ACT = list(range(15, 17))         # Pool SBUF port contends with DVE)

_CACHE = {}


def _build():
    import concourse.bacc as bacc
    import concourse.mybir as mybir
    from concourse import tile

    f32, f16 = mybir.dt.float32, mybir.dt.float16
    A = mybir.AluOpType
    ACT_ID = mybir.ActivationFunctionType.Identity

    nc = bacc.Bacc("TRN2", target_bir_lowering=False, debug=False)

    feat_e = nc.dram_tensor("feat", [PCH, NCH, NB], f32, kind="ExternalInput").ap()
    w0_e = nc.dram_tensor("w0T", [PCH, NCH, HID], f16, kind="ExternalInput").ap()
    w1_e = nc.dram_tensor("w1T", [HID, HID], f16, kind="ExternalInput").ap()
    w2_e = nc.dram_tensor("w2T", [HID, HID], f16, kind="ExternalInput").ap()
    w3_e = nc.dram_tensor("w3T", [HID, NCH, PCH], f16, kind="ExternalInput").ap()
    # biases+corrections, one f32 tensor: col 0..2 = layers 0..2 [128];
    # cols 3..9 = layer-3 chunks [112, 7] (rows 112+ pad)
    bt_e = nc.dram_tensor("bt", [HID, 3 + NCH], f32, kind="ExternalInput").ap()
    out_e = nc.dram_tensor("out", [PCH, NCH, NB], f16, kind="ExternalOutput").ap()

    with tile.TileContext(nc) as tc:
        with (
            tc.tile_pool(name="sbuf", bufs=1) as sb,
            tc.tile_pool(name="psumh", bufs=1, space="PSUM") as psh,
            tc.tile_pool(name="psum3", bufs=6, space="PSUM") as ps3,
        ):
            # ---- loads (host pre-transposed; all partition-contiguous) ----
            feat = sb.tile([PCH, NCH, NB], f32, tag="feat")
            nc.sync.dma_start(feat[:, 0:FEAT_SPLIT, :], feat_e[:, 0:FEAT_SPLIT, :])
            # feat second half rides the scalar queue FIRST (parallel DMA
            # startup with featA); then bt + w0 (short) so the ACT engine is
            # free for its F-multiply share; the rest on sync behind featA.
            nc.scalar.dma_start(
                feat[:, FEAT_SPLIT:NCH, :], feat_e[:, FEAT_SPLIT:NCH, :]
            )
            bt = sb.tile([HID, 3 + NCH], f32, tag="bt")
            nc.scalar.dma_start(bt[:], bt_e[:])
            w0s = sb.tile([PCH, NCH, HID], f16, tag="w0")
            nc.scalar.dma_start(w0s[:], w0_e[:])
            w1s = sb.tile([HID, HID], f16, tag="w1")
            nc.sync.dma_start(w1s[:], w1_e[:])
            w2s = sb.tile([HID, HID], f16, tag="w2")
            nc.sync.dma_start(w2s[:], w2_e[:])
            w3s = sb.tile([HID, NCH, PCH], f16, tag="w3")
            nc.sync.dma_start(w3s[:], w3_e[:])

            b0s = bt[:, 0:1]
            b1s = bt[:, 1:2]
            b2s = bt[:, 2:3]

            # ---- constants (Pool memsets; all early, off critical path) ----
            # scan data1: 65504 pass-through, 1024 at each chain's dummy slot
            maskR = sb.tile([HID, HB, SLOT], f16, tag="maskR")
            nc.gpsimd.memset(maskR[:], FMAX16)
            nc.gpsimd.memset(maskR[:, :, NS:SLOT], OFF_C)
            # gc tiles: dummy slot must hold a harmless finite value (0)
            gc_tiles = {}
            for lname in ("00", "01", "10", "11", "20", "21"):
                g = sb.tile([HID, HB, SLOT], f16, tag=f"gc{lname}")
                nc.gpsimd.memset(g[:, :, NS:SLOT], 0.0)
                gc_tiles[lname] = g
            # per-k F-multiply constants for the ACT ops (activation bias must
            # be an AP; the const-AP pool only carries 0.0/1.0)
            fbias = sb.tile([PCH, len(F_ACT)], f32, tag="fbias")
            for i, k in enumerate(F_ACT):
                nc.gpsimd.memset(fbias[:, i : i + 1], OFF_F - 96.0 * k - C35_64)

            # ---- digitize: 2 DVE ops via f16-at-1536 ----
            # t = f16(16 f - 0.5 + 2^-17 + 1536) == m + 1536 (integer, exact)
            # a' = max(t, 1537)/16 = a + 96 (exact on the 1/16 grid in f16)
            t1 = sb.tile([PCH, NCH, NB], f16, tag="dig1")
            a16 = sb.tile([PCH, NCH, NB], f16, tag="a16")
            for lo, hi in ((0, FEAT_SPLIT), (FEAT_SPLIT, NCH)):
                nc.vector.tensor_scalar(
                    t1[:, lo:hi, :], feat[:, lo:hi, :],
                    16.0, -0.5 + 2.0 ** -17 + F16OFF, A.mult, A.add,
                )
            nc.vector.tensor_scalar(
                a16[:], t1[:], F16OFF + 1.0, 1.0 / 16.0, A.max, A.mult,
            )

            # ---- F' multiplies: F'_k = f16(k*a' + (OFF_F - 96k - 35/64))
            # = round(k*a - 35/64) + OFF_F exactly (zone [1535.4, 1552] has
            # f16 ULP=1; fp32 pre-write value exact on the 1/64 grid).
            # Split across DVE / ACT / GPSIMD.
            F = sb.tile([PCH, NS, NCH, NB], f16, tag="F")

            def f_const(k):
                return OFF_F - 96.0 * k - C35_64

            for lo, hi in ((0, FEAT_SPLIT), (FEAT_SPLIT, NCH)):
                for k in F_DVE:
                    nc.vector.tensor_scalar(
                        F[:, k - 1, lo:hi, :], a16[:, lo:hi, :],
                        float(k), f_const(k), A.mult, A.add,
                    )
            for i, k in enumerate(F_ACT):
                nc.scalar.activation(
                    F[:, k - 1], a16[:], ACT_ID,
                    bias=fbias[:, i : i + 1], scale=float(k),
                )

            # ---- layer 0: D0' = W0 @ F' per lane, psum (k, j); chunk-major
            # accumulation; one group per lane so lane 0 closes early ----
            D0L = []
            for ln in range(2):
                D = psh.tile([HID, HB * NS], f32, tag=f"dh{ln}")
                for c in range(NCH):
                    nc.tensor.matmul(
                        D[:],
                        w0s[:, c, :],
                        F[:, :, c, ln * HB : (ln + 1) * HB],
                        start=(c == 0),
                        stop=(c == NCH - 1),
                    )
                D0L.append(D)
            # (chunk order within each lane already matches the F half split:
            # chunks 0-3 only need the half-A F ops + the two full ACT ops)

            def act_scan(din, bias, lname):
                # ACT: psum -> gc f16 in chain order; the f16 write rounds
                # (bias carries +1024 and all corrections). Then the scan:
                # state = min(max(gc, state), data1); reset to 1024 at dummies.
                gc = gc_tiles[lname]
                nc.scalar.activation(gc[:, :, 0:NS], din, ACT_ID, bias=bias, scale=1.0)
                C = sb.tile([HID, HB, SLOT], f16, tag=f"C{lname}")
                nc.vector.tensor_tensor_scan(
                    C[:].rearrange("p j s -> p (j s)"),
                    gc[:].rearrange("p j s -> p (j s)"),
                    maskR[:].rearrange("p j s -> p (j s)"),
                    OFF_C,
                    A.max,
                    A.min,
                )
                return C

            C0 = [None, None]
            for ln in range(2):
                # D0 psum is (k, j): ACT reads it permuted to (j, k)
                C0[ln] = act_scan(
                    D0L[ln][:].rearrange("p (k j) -> p j k", j=HB), b0s, f"0{ln}"
                )

            def hidden_lane(Cin, w, bias, lname, tag):
                D = psh.tile([HID, HB * NS], f32, tag=tag)
                nc.tensor.matmul(D[:], w[:], Cin[:, :, 0:NS], start=True, stop=True)
                # psum is already (j, k) = chain order
                return act_scan(
                    D[:].rearrange("p (j k) -> p j k", k=NS), bias, lname
                )

            C1 = [None, None]
            C2 = [None, None]
            for ln in range(2):
                C1[ln] = hidden_lane(C0[ln], w1s, b1s, f"1{ln}", f"dh{ln}")
            for ln in range(2):
                C2[ln] = hidden_lane(C1[ln], w2s, b2s, f"2{ln}", f"dh{ln}")

            # ---- layer 3 (output): per chunk, two lane matmuls -> ACT copies
            # psum -> H f16 (+bias, the f16 write rounds) -> DVE max-reduce.
            # round/relu/-1024 all commute with the max; relu on host. ----
            H = sb.tile([PCH, NCH, NB, NS], f16, tag="H")
            Mx = sb.tile([PCH, NCH, NB], f16, tag="mx")
            D3L = []
            for c in range(NCH):
                D3 = ps3.tile([PCH, NB * NS], f32, tag="d3")
                nc.tensor.matmul(
                    D3[:, 0 : HB * NS], w3s[:, c, :], C2[0][:, :, 0:NS],
                    start=True, stop=True,
                )
                D3L.append(D3)
            for c in range(NCH):
                D3 = D3L[c]
                nc.tensor.matmul(
                    D3[:, HB * NS : NB * NS], w3s[:, c, :], C2[1][:, :, 0:NS],
                    start=True, stop=True,
                )
                nc.scalar.activation(
                    H[:, c],
                    D3[:].rearrange("p (j k) -> p j k", k=NS),
                    ACT_ID,
                    bias=bt[0:PCH, 3 + c : 4 + c],
                    scale=1.0,
                )
                nc.vector.tensor_reduce(
                    Mx[:, c, :], H[:, c], mybir.AxisListType.X, A.max
                )

            nc.sync.dma_start(out_e[:, 0:4, :], Mx[:, 0:4, :])
            nc.sync.dma_start(out_e[:, 4:NCH, :], Mx[:, 4:NCH, :])

    nc.compile()
    return nc


def _get_nc():
    if "nc" not in _CACHE:
        _CACHE["nc"] = _build()
    return _CACHE["nc"]


def _prep_in_maps(features, W0, b0, W1, b1, W2, b2, W3, b3):
    f32, f16 = np.float32, np.float16
    w0h = W0.astype(f16)
    w1h = W1.astype(f16)
    w2h = W2.astype(f16)
    w3h = W3.astype(f16)
    w0T = np.ascontiguousarray(
        w0h.T.reshape(NCH, PCH, HID).transpose(1, 0, 2)
    )  # [112, 7, 128]; w0T[p,c,m] = W0[m, c*112+p]
    w1T = np.ascontiguousarray(w1h.T)
    w2T = np.ascontiguousarray(w2h.T)
    w3T = np.ascontiguousarray(w3h.T.reshape(HID, NCH, PCH))

    # bias columns: b - 1/2 + OFF_C - OFF_rhs * rowsum(W_f16), in f64 -> f32.
    # (the 2^-18 ceil-strictness delta of v4 is below the f32 ULP at 1024 and
    # is dropped; threshold-exact cases are measure-zero in fp32.)
    def bias_col(b, wh, off_rhs):
        rs = wh.astype(np.float64).sum(axis=1)
        return b.astype(np.float64) - 0.5 + OFF_C - off_rhs * rs

    bt = np.zeros((HID, 3 + NCH), dtype=f32)
    bt[:, 0] = bias_col(b0, w0h, OFF_F).astype(f32)
    bt[:, 1] = bias_col(b1, w1h, OFF_C).astype(f32)
    bt[:, 2] = bias_col(b2, w2h, OFF_C).astype(f32)
    bt[0:PCH, 3:] = bias_col(b3, w3h, OFF_C).astype(f32).reshape(NCH, PCH).T

    in_maps = []
    for i in range(NCORES):
        shard = features[i * NB : (i + 1) * NB].astype(f32)  # [32, 784]
        feat = np.ascontiguousarray(
            shard.reshape(NB, NCH, PCH).transpose(2, 1, 0)
        )  # [112, 7, 32]
        in_maps.append(
            {
                "feat": feat,
                "w0T": w0T,
                "w1T": w1T,
                "w2T": w2T,
                "w3T": w3T,
                "bt": bt,
            }
        )
    return in_maps


def _assemble(results):
    outs = []
    for i in range(NCORES):
        o = results[i]["out"].astype(np.float32)
        # host: relu(x - 1024)/16 (commutes with the on-device max-reduce)
        o = np.maximum(o - np.float32(OFF_C), 0.0) * np.float32(1.0 / 16.0)
        outs.append(o.transpose(2, 1, 0).reshape(NB, IN))  # [32, 784]
    return np.concatenate(outs, axis=0)


def kernel(features, W0, b0, W1, b1, W2, b2, W3, b3, _trace=False):
    import time
    from concourse.bass_utils import run_bass_kernel_spmd

    nc = _get_nc()
    in_maps = _prep_in_maps(features, W0, b0, W1, b1, W2, b2, W3, b3)
    # The axon-tunneled device occasionally reports a transient
    # NRT_EXEC_UNIT_UNRECOVERABLE on the first attempt after a prior process
    # exited; it recovers on retry.
    last_exc = None
    for attempt in range(3):
        try:
            res = run_bass_kernel_spmd(nc, in_maps, list(range(NCORES)), trace=_trace)
            break
        except Exception as e:  # noqa: BLE001
            last_exc = e
            time.sleep(10 * (attempt + 1))
    else:
        raise last_exc
    out = _assemble(res.results)
    if _trace:
        _CACHE["last_result"] = res
    return out
